# revision 3
# baseline (speedup 1.0000x reference)
"""ClusterGCN layer on 8 TRN2 NeuronCores.

Math: for each cluster c (only intra-cluster edges matter),
    Y_c = B_c @ (X_c @ W) + b
where B_c[d, s] = sum_{edges s->d in c} rsqrt(deg[s])*rsqrt(deg[d])
                  + (1/deg[d]) * [d == s]          (self-loop term)
and deg counts intra-cluster in-edges + 1. Clusters with no intra edge
pass X through unchanged (patched on host).

Host: builds B^T blocks, X^T shards, scatter/gather index maps.
Device (per core): step1 xW = X @ W via PE (fp32r), step2 Y_c = B_c @ xW
as dense [CAP, CAP] x [CAP, F] block matmuls, bias add on DVE.
"""

import os

import numpy as np

N_CORES = 8
N_CLUSTERS = 100
P = 128

_BT_BF16 = os.environ.get("KBT_DTYPE", "bf16") == "bf16"
_XW_BF16 = _BT_BF16  # step-2 operands must share a dtype
_X_BF16 = os.environ.get("KX_DTYPE", "f32") == "bf16"

_prog_cache: dict = {}


def _build_program(cpc: int, cap: int, in_c: int, f_out: int):
    """Build + compile the per-core Bass program.

    cpc: clusters per core; cap: padded cluster size (multiple of 128).
    """
    import concourse.mybir as mybir
    import concourse.tile as tile
    from concourse import bacc

    key = (cpc, cap, in_c, f_out, _BT_BF16, _X_BF16)
    if key in _prog_cache:
        return _prog_cache[key]

    nodes = cpc * cap
    nt = nodes // P          # node tiles of 128
    kc = in_c // P           # contraction chunks for X @ W
    sch = cap // P           # s/d chunks per cluster
    f32 = mybir.dt.float32
    f32r = mybir.dt.float32r
    bf16 = mybir.dt.bfloat16
    # fp32 operands run the PE at 1/4 rate; fp32r is full rate at N>=256.
    # The BIR verifier requires fp32r-consumed buffers be *produced* as
    # fp32r, so the DRAM params and SBUF tiles are declared fp32r outright.
    x_dt = bf16 if _X_BF16 else f32r
    bt_dt = bf16 if _BT_BF16 else f32r
    xw_dt = bf16 if _XW_BF16 else f32r

    nc = bacc.Bacc("TRN2", target_bir_lowering=False, debug=False,
                   num_devices=N_CORES)

    XT = nc.dram_tensor("XT", [in_c, nodes], x_dt, kind="ExternalInput")
    Wt = nc.dram_tensor("Wt", [in_c, f_out], x_dt, kind="ExternalInput")
    BT = nc.dram_tensor("BT", [cpc, cap, cap], bt_dt, kind="ExternalInput")
    BI = nc.dram_tensor("BI", [P, f_out], f32, kind="ExternalInput")
    Y = nc.dram_tensor("Y", [nodes, f_out], f32, kind="ExternalOutput")

    with tile.TileContext(nc) as tc:
        with (
            tc.tile_pool(name="xt", bufs=1) as xt_pool,
            tc.tile_pool(name="w", bufs=1) as w_pool,
            tc.tile_pool(name="xw", bufs=nt) as xw_pool,
            tc.tile_pool(name="bt", bufs=3) as bt_pool,
            tc.tile_pool(name="out", bufs=6) as out_pool,
            tc.tile_pool(name="ps1", bufs=4, space="PSUM") as ps1_pool,
            tc.tile_pool(name="ps2", bufs=4, space="PSUM") as ps2_pool,
        ):
            xt = xt_pool.tile([P, kc, nodes], x_dt)
            nc.sync.dma_start(xt[:], XT.rearrange("(k p) n -> p k n", p=P))
            wt = w_pool.tile([P, kc, f_out], x_dt)
            nc.sync.dma_start(wt[:], Wt.rearrange("(k p) f -> p k f", p=P))
            bi = w_pool.tile([P, f_out], f32)
            nc.sync.dma_start(bi[:], BI[:])

            # step 1: xW node-tile at a time; nodes on partitions
            xw_tiles = []
            for t in range(nt):
                ps = ps1_pool.tile([P, f_out], f32)
                for k in range(kc):
                    nc.tensor.matmul(
                        ps[:],
                        lhsT=xt[:, k, t * P:(t + 1) * P],
                        rhs=wt[:, k, :],
                        start=(k == 0),
                        stop=(k == kc - 1),
                    )
                xw = xw_pool.tile([P, f_out], xw_dt)
                nc.vector.tensor_copy(xw[:], ps[:])
                xw_tiles.append(xw)

            # step 2: per cluster dense aggregation
            for c in range(cpc):
                bt = bt_pool.tile([P, sch, cap], bt_dt)
                nc.sync.dma_start(
                    bt[:], BT[c].rearrange("(so p) d -> p so d", p=P)
                )
                for d in range(sch):
                    ps = ps2_pool.tile([P, f_out], f32)
                    for s in range(sch):
                        nc.tensor.matmul(
                            ps[:],
                            lhsT=bt[:, s, d * P:(d + 1) * P],
                            rhs=xw_tiles[c * sch + s][:],
                            start=(s == 0),
                            stop=(s == sch - 1),
                        )
                    ot = out_pool.tile([P, f_out], f32)
                    nc.vector.tensor_add(out=ot[:], in0=ps[:], in1=bi[:])
                    row = c * cap + d * P
                    nc.sync.dma_start(Y[row:row + P, :], ot[:])

    nc.compile()
    _prog_cache[key] = nc
    return nc


def _host_prep(X, W, b, assign, full_ei):
    """Shard + preprocess. Returns (in_maps, gather info)."""
    n, in_c = X.shape
    f_out = W.shape[1]
    src = full_ei[0].astype(np.int64)
    dst = full_ei[1].astype(np.int64)
    a_s = assign[src]
    intra = a_s == assign[dst]
    es, ed = src[intra], dst[intra]

    deg = np.ones(n, np.float32)
    np.add.at(deg, ed, np.float32(1))
    dis = 1.0 / np.sqrt(deg)

    has_edge = np.zeros(N_CLUSTERS, bool)
    has_edge[np.unique(a_s[intra])] = True

    sizes = np.bincount(assign, minlength=N_CLUSTERS)
    cpc = -(-N_CLUSTERS // N_CORES)            # clusters per core
    cap = max(512, int(-(-sizes.max() // P)) * P)  # padded cluster size

    starts = np.zeros(N_CLUSTERS + 1, np.int64)
    starts[1:] = np.cumsum(sizes)
    order = np.argsort(assign, kind="stable")
    pos = np.empty(n, np.int64)
    pos[order] = np.arange(n) - starts[assign[order]]

    ctot = cpc * N_CORES
    # B^T blocks: Bt[c][s, d] = B_c[d, s]
    Bt = np.zeros((ctot, cap, cap), np.float32)
    np.add.at(Bt, (assign[es], pos[es], pos[ed]),
              (dis[es] * dis[ed]).astype(np.float32))
    Bt[assign, pos, pos] += (1.0 / deg)

    Xp = np.zeros((ctot, cap, in_c), np.float32)
    Xp[assign, pos] = X
    XT_all = np.ascontiguousarray(Xp.reshape(ctot * cap, in_c).T)

    x_np = np.float32
    bt_np = np.float32
    if _X_BF16 or _BT_BF16:
        import ml_dtypes
        if _X_BF16:
            x_np = ml_dtypes.bfloat16
        if _BT_BF16:
            bt_np = ml_dtypes.bfloat16

    bias = np.broadcast_to(b.astype(np.float32), (P, f_out)).copy()
    nodes = cpc * cap
    in_maps = []
    for i in range(N_CORES):
        in_maps.append({
            "XT": np.ascontiguousarray(
                XT_all[:, i * nodes:(i + 1) * nodes]).astype(x_np),
            "Wt": W.astype(np.float32).astype(x_np),
            "BT": Bt[i * cpc:(i + 1) * cpc].astype(bt_np),
            "BI": bias,
        })
    return in_maps, (cpc, cap, has_edge, pos)


def _run(inputs, trace=False, tmpdir=None):
    from concourse.bass_utils import run_bass_kernel_spmd

    X = np.asarray(inputs["X"], np.float32)
    W = np.asarray(inputs["W"], np.float32)
    b = np.asarray(inputs["b"], np.float32)
    assign = np.asarray(inputs["assign"])
    full_ei = np.asarray(inputs["full_ei"])

    n, in_c = X.shape
    f_out = W.shape[1]
    in_maps, (cpc, cap, has_edge, pos) = _host_prep(X, W, b, assign, full_ei)
    nc = _build_program(cpc, cap, in_c, f_out)

    res = run_bass_kernel_spmd(
        nc, in_maps, core_ids=list(range(N_CORES)),
        trace=trace, tmpdir=tmpdir,
    )
    Ydev = np.stack([res.results[i]["Y"] for i in range(N_CORES)])

    c = assign.astype(np.int64)
    core = c // cpc
    row = (c % cpc) * cap + pos
    Y = Ydev[core, row].astype(np.float32)
    miss = ~has_edge[c]
    if miss.any():
        Y[miss] = X[miss]
    return Y, res


def kernel(**inputs) -> np.ndarray:
    Y, _ = _run(inputs)
    return Y


# revision 4
# speedup vs baseline: 1.0205x; 1.0205x over previous
"""ClusterGCN layer on 8 TRN2 NeuronCores.

Math: for each cluster c (only intra-cluster edges matter),
    Y_c = B_c @ (X_c @ W) + b
where B_c[d, s] = sum_{edges s->d in c} rsqrt(deg[s])*rsqrt(deg[d])
                  + (1/deg[d]) * [d == s]          (self-loop term)
and deg counts intra-cluster in-edges + 1. Clusters with no intra edge
pass X through unchanged (patched on host, as is the bias add).

Host: builds B^T blocks, X^T shards, scatter/gather index maps.
Device (per core), pipelined over clusters: step1 xW = X @ W, step2
Y_c = B_c @ xW as dense [CAP, CAP] x [CAP, F] block matmuls on the PE.
"""

import os

import numpy as np

N_CORES = 8
N_CLUSTERS = 100
P = 128

_BT_BF16 = os.environ.get("KBT_DTYPE", "bf16") == "bf16"
_XW_BF16 = _BT_BF16  # step-2 operands must share a dtype
_X_BF16 = os.environ.get("KX_DTYPE", "bf16") == "bf16"

_prog_cache: dict = {}


def _build_program(cpc: int, cap: int, in_c: int, f_out: int):
    """Build + compile the per-core Bass program.

    cpc: clusters per core; cap: padded cluster size (multiple of 128).
    """
    import concourse.mybir as mybir
    import concourse.tile as tile
    from concourse import bacc

    key = (cpc, cap, in_c, f_out, _BT_BF16, _X_BF16)
    if key in _prog_cache:
        return _prog_cache[key]

    kc = in_c // P           # contraction chunks for X @ W
    sch = cap // P           # s/d chunks per cluster
    f32 = mybir.dt.float32
    f32r = mybir.dt.float32r
    bf16 = mybir.dt.bfloat16
    # fp32 operands run the PE at 1/4 rate; fp32r is full rate at N>=256.
    # The BIR verifier requires fp32r-consumed buffers be *produced* as
    # fp32r, so the DRAM params and SBUF tiles are declared fp32r outright.
    x_dt = bf16 if _X_BF16 else f32r
    bt_dt = bf16 if _BT_BF16 else f32r
    xw_dt = bf16 if _XW_BF16 else f32r

    nc = bacc.Bacc("TRN2", target_bir_lowering=False, debug=False,
                   num_devices=N_CORES)

    XT = nc.dram_tensor("XT", [in_c, cpc * cap], x_dt, kind="ExternalInput")
    Wt = nc.dram_tensor("Wt", [in_c, f_out], x_dt, kind="ExternalInput")
    BT = nc.dram_tensor("BT", [cpc, cap, cap], bt_dt, kind="ExternalInput")
    Y = nc.dram_tensor("Y", [cpc * cap, f_out], f32, kind="ExternalOutput")

    XTr = XT.rearrange("(k p) n -> p k n", p=P)

    with tile.TileContext(nc) as tc:
        with (
            tc.tile_pool(name="w", bufs=1) as w_pool,
            tc.tile_pool(name="xt", bufs=3) as xt_pool,
            tc.tile_pool(name="bt", bufs=3) as bt_pool,
            tc.tile_pool(name="xw", bufs=3 * sch) as xw_pool,
            tc.tile_pool(name="out", bufs=8) as out_pool,
            tc.tile_pool(name="ps1", bufs=4, space="PSUM") as ps1_pool,
            tc.tile_pool(name="ps2", bufs=4, space="PSUM") as ps2_pool,
        ):
            wt = w_pool.tile([P, kc, f_out], x_dt)
            nc.sync.dma_start(wt[:], Wt.rearrange("(k p) f -> p k f", p=P))

            for c in range(cpc):
                xt = xt_pool.tile([P, kc, cap], x_dt)
                nc.sync.dma_start(xt[:], XTr[:, :, c * cap:(c + 1) * cap])
                bt = bt_pool.tile([P, sch, cap], bt_dt)
                nc.sync.dma_start(
                    bt[:], BT[c].rearrange("(so p) d -> p so d", p=P)
                )

                # step 1: xW for this cluster's nodes (nodes on partitions)
                xw_tiles = []
                for t in range(sch):
                    ps = ps1_pool.tile([P, f_out], f32)
                    for k in range(kc):
                        nc.tensor.matmul(
                            ps[:],
                            lhsT=xt[:, k, t * P:(t + 1) * P],
                            rhs=wt[:, k, :],
                            start=(k == 0),
                            stop=(k == kc - 1),
                        )
                    xw = xw_pool.tile([P, f_out], xw_dt)
                    nc.vector.tensor_copy(xw[:], ps[:])
                    xw_tiles.append(xw)

                # step 2: dense aggregation for this cluster
                for d in range(sch):
                    ps = ps2_pool.tile([P, f_out], f32)
                    for s in range(sch):
                        nc.tensor.matmul(
                            ps[:],
                            lhsT=bt[:, s, d * P:(d + 1) * P],
                            rhs=xw_tiles[s][:],
                            start=(s == 0),
                            stop=(s == sch - 1),
                        )
                    ot = out_pool.tile([P, f_out], f32)
                    nc.scalar.copy(ot[:], ps[:])
                    row = c * cap + d * P
                    nc.sync.dma_start(Y[row:row + P, :], ot[:])

    nc.compile()
    _prog_cache[key] = nc
    return nc


def _host_prep(X, W, b, assign, full_ei):
    """Shard + preprocess. Returns (in_maps, gather info)."""
    n, in_c = X.shape
    f_out = W.shape[1]
    src = full_ei[0].astype(np.int64)
    dst = full_ei[1].astype(np.int64)
    a_s = assign[src]
    intra = a_s == assign[dst]
    es, ed = src[intra], dst[intra]

    deg = np.ones(n, np.float32)
    np.add.at(deg, ed, np.float32(1))
    dis = 1.0 / np.sqrt(deg)

    has_edge = np.zeros(N_CLUSTERS, bool)
    has_edge[np.unique(a_s[intra])] = True

    sizes = np.bincount(assign, minlength=N_CLUSTERS)
    cpc = -(-N_CLUSTERS // N_CORES)            # clusters per core
    cap = max(512, int(-(-sizes.max() // P)) * P)  # padded cluster size

    starts = np.zeros(N_CLUSTERS + 1, np.int64)
    starts[1:] = np.cumsum(sizes)
    order = np.argsort(assign, kind="stable")
    pos = np.empty(n, np.int64)
    pos[order] = np.arange(n) - starts[assign[order]]

    ctot = cpc * N_CORES
    # B^T blocks: Bt[c][s, d] = B_c[d, s]
    Bt = np.zeros((ctot, cap, cap), np.float32)
    np.add.at(Bt, (assign[es], pos[es], pos[ed]),
              (dis[es] * dis[ed]).astype(np.float32))
    Bt[assign, pos, pos] += (1.0 / deg)

    Xp = np.zeros((ctot, cap, in_c), np.float32)
    Xp[assign, pos] = X
    XT_all = np.ascontiguousarray(Xp.reshape(ctot * cap, in_c).T)

    x_np = np.float32
    bt_np = np.float32
    if _X_BF16 or _BT_BF16:
        import ml_dtypes
        if _X_BF16:
            x_np = ml_dtypes.bfloat16
        if _BT_BF16:
            bt_np = ml_dtypes.bfloat16

    nodes = cpc * cap
    in_maps = []
    for i in range(N_CORES):
        in_maps.append({
            "XT": np.ascontiguousarray(
                XT_all[:, i * nodes:(i + 1) * nodes]).astype(x_np),
            "Wt": W.astype(np.float32).astype(x_np),
            "BT": Bt[i * cpc:(i + 1) * cpc].astype(bt_np),
        })
    return in_maps, (cpc, cap, has_edge, pos)


def _run(inputs, trace=False, tmpdir=None):
    from concourse.bass_utils import run_bass_kernel_spmd

    X = np.asarray(inputs["X"], np.float32)
    W = np.asarray(inputs["W"], np.float32)
    b = np.asarray(inputs["b"], np.float32)
    assign = np.asarray(inputs["assign"])
    full_ei = np.asarray(inputs["full_ei"])

    n, in_c = X.shape
    f_out = W.shape[1]
    in_maps, (cpc, cap, has_edge, pos) = _host_prep(X, W, b, assign, full_ei)
    nc = _build_program(cpc, cap, in_c, f_out)

    res = run_bass_kernel_spmd(
        nc, in_maps, core_ids=list(range(N_CORES)),
        trace=trace, tmpdir=tmpdir,
    )
    Ydev = np.stack([res.results[i]["Y"] for i in range(N_CORES)])

    c = assign.astype(np.int64)
    core = c // cpc
    row = (c % cpc) * cap + pos
    Y = Ydev[core, row] + b[None, :].astype(np.float32)
    miss = ~has_edge[c]
    if miss.any():
        Y[miss] = X[miss]
    return Y, res


def kernel(**inputs) -> np.ndarray:
    Y, _ = _run(inputs)
    return Y


# revision 5
# speedup vs baseline: 1.5789x; 1.5472x over previous
"""ClusterGCN layer on 8 TRN2 NeuronCores.

Math: for each cluster c (only intra-cluster edges matter),
    Y_c = B_c @ (X_c @ W) + b
where B_c[d, s] = sum_{edges s->d in c} rsqrt(deg[s])*rsqrt(deg[d])
                  + (1/deg[d]) * [d == s]          (self-loop term)
and deg counts intra-cluster in-edges + 1. Clusters with no intra edge
pass X through unchanged (patched on host, as is the bias add).

Host: builds B^T blocks, X^T shards, scatter/gather index maps.
Device (per core), pipelined over clusters:
  step1: xW = X @ W       (lhsT = X^T chunks, rhs = W, N=256)
  step2: Y_c^T = (B_c @ xW)^T via lhsT = xW chunks (stationary),
         rhs = B_c^T rows (moving, N=cap), PSUM accumulated over s.
Output is per-cluster transposed [f, cap]; host de-transposes in the
gather (free), which lets step2 stream N=512-wide matmuls.
"""

import os

import numpy as np

N_CORES = 8
N_CLUSTERS = 100
P = 128

_BT_BF16 = os.environ.get("KBT_DTYPE", "bf16") == "bf16"
_XW_BF16 = _BT_BF16  # step-2 operands must share a dtype
_X_BF16 = os.environ.get("KX_DTYPE", "bf16") == "bf16"

_prog_cache: dict = {}


def _build_program(cpc: int, cap: int, in_c: int, f_out: int):
    """Build + compile the per-core Bass program.

    cpc: clusters per core; cap: padded cluster size (multiple of 128).
    """
    import concourse.mybir as mybir
    import concourse.tile as tile
    from concourse import bacc

    key = (cpc, cap, in_c, f_out, _BT_BF16, _X_BF16)
    if key in _prog_cache:
        return _prog_cache[key]

    kc = in_c // P           # contraction chunks for X @ W
    sch = cap // P           # s chunks per cluster
    fc = f_out // P          # f chunks (step-2 output partitions)
    f32 = mybir.dt.float32
    f32r = mybir.dt.float32r
    bf16 = mybir.dt.bfloat16
    # fp32 matmul operands run the PE at 1/4 rate; fp32r is full rate at
    # N>=256 but must be produced as fp32r end-to-end for the verifier.
    x_dt = bf16 if _X_BF16 else f32r
    bt_dt = bf16 if _BT_BF16 else f32r
    xw_dt = bf16 if _XW_BF16 else f32r

    XG = 4                   # clusters per XT load
    BG = 2                   # clusters per BT load

    nc = bacc.Bacc("TRN2", target_bir_lowering=False, debug=False,
                   num_devices=N_CORES)

    XT = nc.dram_tensor("XT", [in_c, cpc * cap], x_dt, kind="ExternalInput")
    Wt = nc.dram_tensor("Wt", [in_c, f_out], x_dt, kind="ExternalInput")
    BT = nc.dram_tensor("BT", [cpc, cap, cap], bt_dt, kind="ExternalInput")
    YT = nc.dram_tensor("YT", [cpc, f_out, cap], f32, kind="ExternalOutput")

    XTr = XT.rearrange("(k p) n -> p k n", p=P)

    with tile.TileContext(nc) as tc:
        with (
            tc.tile_pool(name="w", bufs=1) as w_pool,
            tc.tile_pool(name="xt", bufs=2) as xt_pool,
            tc.tile_pool(name="bt", bufs=3) as bt_pool,
            tc.tile_pool(name="xw", bufs=4 * sch) as xw_pool,
            tc.tile_pool(name="out", bufs=5) as out_pool,
            tc.tile_pool(name="ps1", bufs=4, space="PSUM") as ps1_pool,
            tc.tile_pool(name="ps2", bufs=4, space="PSUM") as ps2_pool,
        ):
            wt = w_pool.tile([P, kc, f_out], x_dt)
            nc.sync.dma_start(wt[:], Wt.rearrange("(k p) f -> p k f", p=P))

            xt = bt = None
            xg_size = bg_size = 0
            for c in range(cpc):
                if c % XG == 0:
                    xg_size = min(XG, cpc - c)
                    xt = xt_pool.tile([P, kc, XG * cap], x_dt)
                    nc.sync.dma_start(
                        xt[:, :, :xg_size * cap],
                        XTr[:, :, c * cap:(c + xg_size) * cap],
                    )
                if c % BG == 0:
                    bg_size = min(BG, cpc - c)
                    bt = bt_pool.tile([P, BG, sch, cap], bt_dt)
                    nc.sync.dma_start(
                        bt[:, :bg_size],
                        BT[c:c + bg_size].rearrange(
                            "c (so p) d -> p c so d", p=P),
                    )
                xoff = (c % XG) * cap
                ci = c % BG

                # step 1: xW for this cluster's nodes (nodes on partitions)
                xw_tiles = []
                for t in range(sch):
                    ps = ps1_pool.tile([P, f_out], f32)
                    for k in range(kc):
                        nc.tensor.matmul(
                            ps[:],
                            lhsT=xt[:, k, xoff + t * P:xoff + (t + 1) * P],
                            rhs=wt[:, k, :],
                            start=(k == 0),
                            stop=(k == kc - 1),
                        )
                    xw = xw_pool.tile([P, f_out], xw_dt)
                    nc.scalar.copy(xw[:], ps[:])
                    xw_tiles.append(xw)

                # step 2: Y_c^T = (B_c @ xW)^T, f on partitions, d free
                ot = out_pool.tile([P, fc, cap], f32)
                for f in range(fc):
                    ps = ps2_pool.tile([P, cap], f32)
                    for s in range(sch):
                        nc.tensor.matmul(
                            ps[:],
                            lhsT=xw_tiles[s][:, f * P:(f + 1) * P],
                            rhs=bt[:, ci, s, :],
                            start=(s == 0),
                            stop=(s == sch - 1),
                        )
                    nc.vector.tensor_copy(ot[:, f, :], ps[:])
                nc.sync.dma_start(
                    YT[c].rearrange("(f p) d -> p f d", p=P), ot[:]
                )

    nc.compile()
    _prog_cache[key] = nc
    return nc


def _host_prep(X, W, b, assign, full_ei):
    """Shard + preprocess. Returns (in_maps, gather info)."""
    n, in_c = X.shape
    f_out = W.shape[1]
    src = full_ei[0].astype(np.int64)
    dst = full_ei[1].astype(np.int64)
    a_s = assign[src]
    intra = a_s == assign[dst]
    es, ed = src[intra], dst[intra]

    deg = np.ones(n, np.float32)
    np.add.at(deg, ed, np.float32(1))
    dis = 1.0 / np.sqrt(deg)

    has_edge = np.zeros(N_CLUSTERS, bool)
    has_edge[np.unique(a_s[intra])] = True

    sizes = np.bincount(assign, minlength=N_CLUSTERS)
    cpc = -(-N_CLUSTERS // N_CORES)            # clusters per core
    cap = max(512, int(-(-sizes.max() // P)) * P)  # padded cluster size

    starts = np.zeros(N_CLUSTERS + 1, np.int64)
    starts[1:] = np.cumsum(sizes)
    order = np.argsort(assign, kind="stable")
    pos = np.empty(n, np.int64)
    pos[order] = np.arange(n) - starts[assign[order]]

    ctot = cpc * N_CORES
    # B^T blocks: Bt[c][s, d] = B_c[d, s]
    Bt = np.zeros((ctot, cap, cap), np.float32)
    np.add.at(Bt, (assign[es], pos[es], pos[ed]),
              (dis[es] * dis[ed]).astype(np.float32))
    Bt[assign, pos, pos] += (1.0 / deg)

    Xp = np.zeros((ctot, cap, in_c), np.float32)
    Xp[assign, pos] = X
    XT_all = np.ascontiguousarray(Xp.reshape(ctot * cap, in_c).T)

    x_np = np.float32
    bt_np = np.float32
    if _X_BF16 or _BT_BF16:
        import ml_dtypes
        if _X_BF16:
            x_np = ml_dtypes.bfloat16
        if _BT_BF16:
            bt_np = ml_dtypes.bfloat16

    nodes = cpc * cap
    in_maps = []
    for i in range(N_CORES):
        in_maps.append({
            "XT": np.ascontiguousarray(
                XT_all[:, i * nodes:(i + 1) * nodes]).astype(x_np),
            "Wt": W.astype(np.float32).astype(x_np),
            "BT": Bt[i * cpc:(i + 1) * cpc].astype(bt_np),
        })
    return in_maps, (cpc, cap, has_edge, pos)


def _run(inputs, trace=False, tmpdir=None):
    from concourse.bass_utils import run_bass_kernel_spmd

    X = np.asarray(inputs["X"], np.float32)
    W = np.asarray(inputs["W"], np.float32)
    b = np.asarray(inputs["b"], np.float32)
    assign = np.asarray(inputs["assign"])
    full_ei = np.asarray(inputs["full_ei"])

    n, in_c = X.shape
    f_out = W.shape[1]
    in_maps, (cpc, cap, has_edge, pos) = _host_prep(X, W, b, assign, full_ei)
    nc = _build_program(cpc, cap, in_c, f_out)

    res = run_bass_kernel_spmd(
        nc, in_maps, core_ids=list(range(N_CORES)),
        trace=trace, tmpdir=tmpdir,
    )
    # YT: [core][cpc, f_out, cap]
    YTdev = np.stack([res.results[i]["YT"] for i in range(N_CORES)])

    c = assign.astype(np.int64)
    core = c // cpc
    lc = c % cpc
    Y = YTdev[core, lc, :, pos] + b[None, :].astype(np.float32)
    miss = ~has_edge[c]
    if miss.any():
        Y[miss] = X[miss]
    return Y, res


def kernel(**inputs) -> np.ndarray:
    Y, _ = _run(inputs)
    return Y
